# revision 33
# baseline (speedup 1.0000x reference)
"""Trainium2 Bass kernel for a 12-head causal attention block (GPT-2 style).

Problem: x:[4,2048,768] -> qkv = x@W_attn+b_attn, causal softmax attention
(12 heads, d=64), out @ W_proj + b_proj.

Sharding over 8 NeuronCores: core c handles batch b=c//2 (data parallel) and
head-group hg=c%2 (6 heads = 3 head-pairs, tensor parallel on the qkv
columns / proj rows).  Each core returns a partial projection output; the
host sums the two head-group partials per batch and adds b_proj.

v2 design (vs the 410us baseline):
  - x is transposed and bf16-cast on the HOST: no PE-transpose phase, half
    the input DMA bytes, and every matmul operand is bf16 (fast weight
    load applies; PSUM accumulation stays fp32).
  - scores: the two heads of a pair run CONCURRENTLY in the PE array via
    row tiling (tile_position (0,0)/(64,0), K=64 each) - halves score time.
  - per-(pair,g) attention group: j-loop over k-tiles software-pipelined
    one stage deep (scores j+1 emitted before AV j) so the ACT exp of tile
    j overlaps the scores matmul of j+1.
  - AV uses the M=65 ones-column trick: attention output AND softmax
    denominators from one accumulating matmul per head.
  - normalization: DVE reciprocal -> GPSIMD partition_broadcast -> DVE
    multiply (no PE broadcast matmul, no PSUM->SBUF bounce of it).
  - qkv / proj matmuls are emitted as small work units INTERLEAVED into the
    attention j-loops: the PE executes them while ACT (the per-group
    bottleneck at ~1 elem/cycle/lane) chews on exp, keeping the PE dense so
    the HAM clock stays at 2.4 GHz.
  - PSUM budget: scores 2x[128,1024] (4 banks) + AV 2x[65,512] (2 banks) +
    shared aux pool 2x[128,512] (2 banks) = 8 banks exactly.
"""

import os
import ml_dtypes
import numpy as np

N_HEAD = 12
N_EMBD = 768
HEAD_DIM = 64
B, S = 4, 2048
N_CORES = 8
HG_HEADS = 6            # heads per core (3 pairs)
HG_DIM = HG_HEADS * HEAD_DIM   # 384
QKV_W = 3 * HG_DIM      # 1152 qkv columns per core
N_PAIRS = 3
ST = S // 128           # 16 seq tiles of 128
NG = S // 512           # 4 seq groups of 512

LAST_RESULTS = None
_PROGRAM = None


def _build_program():
    import concourse.bacc as bacc
    import concourse.tile as tile
    from concourse import mybir

    F32 = mybir.dt.float32
    BF16 = mybir.dt.bfloat16
    AF = mybir.ActivationFunctionType

    nc = bacc.Bacc(None, target_bir_lowering=False)
    xT_d = nc.declare_dram_parameter("xT", [N_EMBD, S], BF16, isOutput=False)
    wqkv_d = nc.declare_dram_parameter("w_qkv", [N_EMBD, QKV_W], BF16, isOutput=False)
    bqk_d = nc.declare_dram_parameter("b_qk", [768], F32, isOutput=False)
    bv_d = nc.declare_dram_parameter("b_v", [HG_DIM], BF16, isOutput=False)
    wproj_d = nc.declare_dram_parameter("w_proj", [HG_DIM, N_EMBD], BF16, isOutput=False)
    ones_d = nc.declare_dram_parameter("ones", [1, 128], BF16, isOutput=False)
    y_d = nc.declare_dram_parameter("y", [S, N_EMBD], F32, isOutput=True)

    with tile.TileContext(nc) as tc:
        from contextlib import ExitStack

        with ExitStack() as outer:
            consts = outer.enter_context(tc.tile_pool(name="consts", bufs=1))
            ones_row = consts.tile([1, 128], BF16)
            nc.sync.dma_start(out=ones_row[:], in_=ones_d[:])
            bias_qk = consts.tile([128, 6], F32)      # col m: b_qk[128m:128m+128]
            nc.sync.dma_start(
                out=bias_qk[:], in_=bqk_d[0:768].rearrange("(m p) -> p m", p=128)
            )
            bias_v = consts.tile([1, HG_DIM], BF16)
            nc.sync.dma_start(
                out=bias_v[:], in_=bv_d[0:HG_DIM].rearrange("(o v) -> o v", o=1)
            )

            # ---- persistent activations/weights in SBUF (all bf16) ----
            big = outer.enter_context(tc.tile_pool(name="big", bufs=1))
            xT = big.tile([128, 6 * S], BF16)       # [emb-part, k-chunk*2048+seq]
            w_all = big.tile([128, 6 * QKV_W], BF16)
            w_proj = big.tile([128, N_PAIRS * N_EMBD], BF16)
            qkT = big.tile([128, 6 * S], BF16)      # m=0..2 qT pairs, m=3..5 kT pairs
            # per k-tile: 6 heads x (64 v-cols + a ones col for the softmax
            # denominator) -> P@V and row-sums come from one M=65 matmul
            v_all = big.tile([128, ST * 390], BF16)  # [seq, t*390 + 65h + d]
            attnT = big.tile([128, N_PAIRS * S], BF16)

            nc.gpsimd.memset(v_all[:], 1.0)
            # Each dma_start costs ~0.6us of ISSUE time on its trigger
            # engine, so the critical first-group inputs (w chunks + xT g0
            # quarters) are issued from TWO engines in parallel - sync and
            # scalar (ACT is idle until the first exp).
            for k in range(6):
                nc.sync.dma_start(out=w_all[:, k * QKV_W:(k + 1) * QKV_W],
                                  in_=wqkv_d[k * 128:(k + 1) * 128, :])
                nc.scalar.dma_start(
                    out=xT[:, k * S:k * S + 512],
                    in_=xT_d[k * 128:(k + 1) * 128, 0:512])
            # later xT quarters and w_proj are DEADLINE-queued (below) so
            # their DMAs queue behind the critical first-group inputs and
            # don't steal lead-in bandwidth.
            def emit_xT_quarter(g):
                for k in range(6):
                    nc.sync.dma_start(
                        out=xT[:, k * S + g * 512:k * S + (g + 1) * 512],
                        in_=xT_d[k * 128:(k + 1) * 128, g * 512:(g + 1) * 512])

            def emit_wproj():
                for p in range(N_PAIRS):
                    nc.sync.dma_start(out=w_proj[:, p * N_EMBD:(p + 1) * N_EMBD],
                                      in_=wproj_d[p * 128:(p + 1) * 128, :])

            # ---- pools ----
            stps = outer.enter_context(tc.tile_pool(name="stps", bufs=2, space="PSUM"))
            avps = outer.enter_context(tc.tile_pool(name="avps", bufs=2, space="PSUM"))
            auxps = outer.enter_context(tc.tile_pool(name="auxps", bufs=2, space="PSUM"))
            ptp = outer.enter_context(tc.tile_pool(name="ptp", bufs=3))
            avsb = outer.enter_context(tc.tile_pool(name="avsb", bufs=4))
            rcp = outer.enter_context(tc.tile_pool(name="rcp", bufs=4))
            bcp = outer.enter_context(tc.tile_pool(name="bcp", bufs=4))
            shtmp = outer.enter_context(tc.tile_pool(name="shtmp", bufs=2))
            ystage = outer.enter_context(tc.tile_pool(name="ystage", bufs=3))

            v_view = v_all[:].rearrange("p (t h c) -> p t h c", t=ST, h=HG_HEADS)

            # ---- work-unit emitters (each emits a small PE-dense chunk) ----
            def emit_qk_group(m, g):
                # qkT[:, m*S + g*512 : +512] = (W[:, m-block].T @ xT)[:, g-block] + bias
                ps = auxps.tile([128, 512], F32, tag="aux")
                for k in range(6):
                    nc.tensor.matmul(
                        ps[:],
                        w_all[:, k * QKV_W + m * 128:k * QKV_W + (m + 1) * 128],
                        xT[:, k * S + g * 512:k * S + (g + 1) * 512],
                        start=(k == 0), stop=(k == 5),
                    )
                nc.vector.tensor_scalar_add(
                    qkT[:, m * S + g * 512:m * S + (g + 1) * 512],
                    ps[:], bias_qk[:, m:m + 1],
                )

            def emit_v_tile(t):
                # v rows t*128.. for all 6 heads (N=384)
                ps = auxps.tile([128, HG_DIM], F32, tag="aux")
                for k in range(6):
                    nc.tensor.matmul(
                        ps[:],
                        xT[:, k * S + t * 128:k * S + (t + 1) * 128],
                        w_all[:, k * QKV_W + 768:k * QKV_W + QKV_W],
                        start=(k == 0), stop=False,
                    )
                nc.tensor.matmul(   # += ones^T[1,128].T @ bias_v[1,384]
                    ps[:], ones_row[:], bias_v[:], start=False, stop=True,
                )
                nc.vector.tensor_copy(
                    v_view[:, t, :, 0:64],
                    ps[:].rearrange("p (h d) -> p h d", h=6),
                )

            def emit_proj_tile(t):
                psA = auxps.tile([128, 512], F32, tag="aux")
                psB = auxps.tile([128, 256], F32, tag="aux")
                for p in range(N_PAIRS):
                    lhsT = attnT[:, p * S + t * 128:p * S + (t + 1) * 128]
                    nc.tensor.matmul(psA[:], lhsT, w_proj[:, p * N_EMBD:p * N_EMBD + 512],
                                     start=(p == 0), stop=(p == N_PAIRS - 1))
                    nc.tensor.matmul(psB[:], lhsT,
                                     w_proj[:, p * N_EMBD + 512:(p + 1) * N_EMBD],
                                     start=(p == 0), stop=(p == N_PAIRS - 1))
                ys = ystage.tile([128, N_EMBD], F32)
                nc.vector.tensor_copy(ys[:, 0:512], psA[:])
                nc.vector.tensor_copy(ys[:, 512:768], psB[:])
                nc.sync.dma_start(out=y_d[t * 128:(t + 1) * 128, :], in_=ys[:])

            # ---- deadline-driven background work queue ----
            # Attention groups execute in a fixed order; (pair, g, j) maps to
            # a global step.  Each qkv/proj work unit carries the step by
            # which it MUST be emitted (Tile deps are emission-order-based:
            # a read emitted before its producer gets no dependency).  Units
            # are pulled with LOOKAHEAD steps of slack so the PE always has
            # background matmuls to chew on while ACT runs exp.
            # pair-2 groups run [1,0,3,2]: each group's proj tiles become
            # valid one group later (after the deferred normalize), so this
            # order leaves only ONE group's proj tiles (t8-11) plus the
            # final normalize as tail work after the last exp.
            group_order = {0: [0, 1, 2, 3], 1: [0, 1, 2, 3], 2: [1, 0, 3, 2]}
            step_base = {}
            _acc = 0
            for _p in range(N_PAIRS):
                for _g in group_order[_p]:
                    step_base[(_p, _g)] = _acc
                    _acc += 4 * _g + 4
            TOTAL_STEPS = _acc
            LOOKAHEAD = 9

            work_q = []   # sorted list of (deadline_step, seq, fn)
            _seq = [0]

            def push(deadline, fn):
                import bisect
                _seq[0] += 1
                bisect.insort(work_q, (deadline, _seq[0], fn))

            def pull_work(cur_step):
                # overdue units MUST emit now (correctness: emission order
                # defines Tile dependencies); otherwise spread at one unit
                # per step so the background work stays evenly interleaved.
                while work_q and work_q[0][0] <= cur_step:
                    work_q.pop(0)[2]()
                if work_q and work_q[0][0] <= cur_step + LOOKAHEAD:
                    work_q.pop(0)[2]()

            # ---- attention group with interleaved background units ----
            def emit_attn_group(pair, g, pre_unit=None):
                """pre_unit: emitted right after the pipeline warm-up, BEFORE
                any work-queue unit (the deferred normalize must precede proj
                units that read the attnT columns it writes)."""
                q0 = pair * S
                k0 = (3 + pair) * S
                njt = 4 * g + 4
                av0 = avps.tile([65, 512], F32, tag="av")
                av1 = avps.tile([65, 512], F32, tag="av")
                sts = {}
                pts = {}

                def scores(j):
                    diag_r = j - 4 * g
                    c0 = 128 * diag_r if diag_r >= 0 else 0
                    st = stps.tile([128, 1024], F32, tag="st")
                    nc.tensor.matmul(
                        st[:, c0:512],
                        qkT[0:64, k0 + j * 128:k0 + (j + 1) * 128],
                        qkT[0:64, q0 + g * 512 + c0:q0 + (g + 1) * 512],
                        start=True, stop=True, tile_position=(0, 0),
                    )
                    nc.tensor.matmul(
                        st[:, 512 + c0:1024],
                        qkT[64:128, k0 + j * 128:k0 + (j + 1) * 128],
                        qkT[64:128, q0 + g * 512 + c0:q0 + (g + 1) * 512],
                        start=True, stop=True, tile_position=(64, 0),
                    )
                    sts[j] = (st, c0)

                def expmask(j):
                    st, c0 = sts.pop(j)
                    pt = ptp.tile([128, 1024], BF16, tag="pt")
                    nc.scalar.activation(pt[:, c0:1024], st[:, c0:1024],
                                         AF.Exp, bias=0.0, scale=0.125)
                    diag_r = j - 4 * g
                    if diag_r >= 0:
                        for h in range(2):
                            nc.gpsimd.affine_select(
                                out=pt[:, h * 512 + c0:h * 512 + c0 + 128],
                                in_=pt[:, h * 512 + c0:h * 512 + c0 + 128],
                                compare_op=mybir.AluOpType.is_ge,
                                fill=0.0, base=0,
                                pattern=[[1, 128]], channel_multiplier=-1,
                            )
                    pts[j] = (pt, c0)

                def av(j):
                    pt, c0 = pts.pop(j)
                    first, last = (j == 0), (j == njt - 1)
                    for h, avt in ((0, av0), (1, av1)):
                        nc.tensor.matmul(
                            avt[0:65, c0:512],
                            v_all[:, j * 390 + (2 * pair + h) * 65:
                                  j * 390 + (2 * pair + h) * 65 + 65],
                            pt[:, h * 512 + c0:(h + 1) * 512],
                            start=first, stop=last,
                        )

                scores(0)
                expmask(0)
                if pre_unit is not None:
                    pre_unit()
                base = step_base[(pair, g)]
                for j in range(njt):
                    if j + 1 < njt:
                        scores(j + 1)
                        expmask(j + 1)
                    pull_work(base + j)
                    av(j)

                # evacuate the AV accumulators to SBUF with one fast copy per
                # head (frees the PSUM banks for the next group's AV almost
                # immediately); the recip/broadcast/multiply chain is DEFERRED
                # into the next group's instruction stream so it never stalls
                # the PE at the group boundary.
                avsb0 = avsb.tile([65, 512], F32, tag="avsb")
                avsb1 = avsb.tile([65, 512], F32, tag="avsb")
                nc.vector.tensor_copy(avsb0[:], av0[:])
                nc.vector.tensor_copy(avsb1[:], av1[:])

                def normalize():
                    cols = slice(pair * S + g * 512, pair * S + (g + 1) * 512)
                    # DVE reciprocal runs ~9 cyc/elem PER LANE: on [1,512] it
                    # costs 3.3us and head-of-line-blocks the DVE queue.
                    # Reshape both heads' denominators to [128,8] via SBUF
                    # DMAs (row-major pairing, probe-verified) so the recip
                    # uses 128 lanes (~0.1us), then shape back for the
                    # gpsimd partition broadcast.
                    dn8 = rcp.tile([128, 8], F32, tag="dn8")
                    nc.sync.dma_start(out=dn8[0:64, :], in_=avsb0[64:65, :])
                    nc.sync.dma_start(out=dn8[64:128, :], in_=avsb1[64:65, :])
                    rc8 = rcp.tile([128, 8], F32, tag="rc8")
                    with nc.allow_low_precision(reason="softmax normalize bf16"):
                        nc.vector.reciprocal(rc8[:], dn8[:])
                        for h, avt in ((0, avsb0), (1, avsb1)):
                            rc = rcp.tile([1, 512], F32, tag="rcrow")
                            nc.sync.dma_start(out=rc[:],
                                              in_=rc8[64 * h:64 * h + 64, :])
                            bc = bcp.tile([64, 512], F32)
                            nc.gpsimd.partition_broadcast(bc[:], rc[:],
                                                          channels=64)
                            if h == 0:
                                nc.vector.tensor_mul(attnT[0:64, cols],
                                                     avt[0:64, :], bc[:])
                            else:
                                # DVE lanes are partition-locked: odd head's
                                # rows 64-127 via an SBUF bounce + DMA shift
                                tmp = shtmp.tile([64, 512], BF16)
                                nc.vector.tensor_mul(tmp[:], avt[0:64, :], bc[:])
                                nc.sync.dma_start(out=attnT[64:128, cols],
                                                  in_=tmp[:])
                return normalize

            # ================= schedule =================
            # upfront: just enough qkv for attn(0, g0); v t0-3 go through
            # the deadline queue (first read at av(j=t) of group (0,0))
            emit_qk_group(3, 0)          # kT pair 0, seq 0-511
            emit_qk_group(0, 0)          # qT pair 0, seq 0-511

            # deadlines: qT(p, g) is read only by group (p, g); kT(p, g') is
            # read by EVERY group (p, g >= g'), so its deadline is the
            # earliest-executing such group - for pair 2 (descending group
            # order) that is the first group of the pair for ALL kT chunks.
            for p in range(N_PAIRS):
                first_step = min(step_base[(p, g)] for g in group_order[p])
                for g in range(NG):
                    if (p, g) == (0, 0):
                        continue
                    kt_dl = min(step_base[(p, gg)] for gg in range(g, NG)) - 1
                    push(kt_dl, lambda m=3 + p, g=g: emit_qk_group(m, g))
                    push(step_base[(p, g)] - 1,
                         lambda m=p, g=g: emit_qk_group(m, g))
            for t in range(16):
                # first pair-0 group reading tile t is g = t//4 (at j = t)
                push(step_base[(0, t // 4)] + t, lambda t=t: emit_v_tile(t))
            # xT quarter g is first read by qk(0, g) units (deadline base-1)
            for g in range(1, NG):
                push(step_base[(0, g)] - 2, lambda g=g: emit_xT_quarter(g))
            # w_proj is first read by proj units in pair 2
            push(step_base[(1, 0)], emit_wproj)

            deferred_norm = None
            for pair in range(N_PAIRS):
                for g in group_order[pair]:
                    deferred_norm = emit_attn_group(pair, g,
                                                    pre_unit=deferred_norm)
                    if pair == 2:
                        # proj tiles for this group's columns become valid
                        # once its (deferred) normalize is emitted - which
                        # happens as the NEXT group's pre_unit (step nxt).
                        # deadline nxt+1+LOOKAHEAD => first pullable at
                        # step nxt+1, strictly after that pre_unit.
                        nxt = step_base[(pair, g)] + 4 * g + 4
                        for t in range(4 * g, 4 * g + 4):
                            push(nxt + 1 + LOOKAHEAD,
                                 lambda t=t: emit_proj_tile(t))

            # drain: last group's normalize + anything not pulled + tail proj
            if deferred_norm is not None:
                deferred_norm()
            while work_q:
                work_q.pop(0)[2]()

    nc.compile()
    return nc


def _numpy_fallback(x, mask, W_attn, b_attn, W_proj, b_proj):
    qkv = x @ W_attn + b_attn
    q, k, v = np.split(qkv, 3, axis=-1)

    def heads(t):
        return t.reshape(B, S, N_HEAD, HEAD_DIM).transpose(0, 2, 1, 3)

    q, k, v = heads(q), heads(k), heads(v)
    attn = np.einsum("bhqd,bhkd->bhqk", q, k) / np.sqrt(np.float32(HEAD_DIM))
    attn = attn + mask * (-1e9)
    attn = attn - attn.max(axis=-1, keepdims=True)
    attn = np.exp(attn)
    attn = attn / attn.sum(axis=-1, keepdims=True)
    out = np.einsum("bhqk,bhkd->bhqd", attn, v)
    out = out.transpose(0, 2, 1, 3).reshape(B, S, N_EMBD)
    return (out @ W_proj + b_proj).astype(np.float32)


def make_in_maps(x, W_attn, b_attn, W_proj):
    bf16 = ml_dtypes.bfloat16
    in_maps = []
    for c in range(N_CORES):
        b, hg = divmod(c, 2)
        o = HG_DIM * hg
        in_maps.append({
            "xT": np.ascontiguousarray(x[b].T.astype(bf16)),
            "w_qkv": np.ascontiguousarray(np.concatenate(
                [W_attn[:, o:o + HG_DIM],
                 W_attn[:, 768 + o:768 + o + HG_DIM],
                 W_attn[:, 1536 + o:1536 + o + HG_DIM]], axis=1).astype(bf16)),
            "b_qk": np.ascontiguousarray(np.concatenate(
                [b_attn[o:o + HG_DIM], b_attn[768 + o:768 + o + HG_DIM]])),
            "b_v": np.ascontiguousarray(b_attn[1536 + o:1536 + o + HG_DIM]).astype(bf16),
            "w_proj": np.ascontiguousarray(W_proj[o:o + HG_DIM, :].astype(bf16)),
            "ones": np.ones((1, 128), dtype=bf16),
        })
    return in_maps


def kernel(x, mask, W_attn, b_attn, W_proj, b_proj):
    global LAST_RESULTS, _PROGRAM
    x = np.asarray(x, dtype=np.float32)
    mask = np.asarray(mask, dtype=np.float32)
    W_attn = np.asarray(W_attn, dtype=np.float32)
    b_attn = np.asarray(b_attn, dtype=np.float32)
    W_proj = np.asarray(W_proj, dtype=np.float32)
    b_proj = np.asarray(b_proj, dtype=np.float32)

    # the kernel exploits causal structure; verify the mask actually is causal
    causal = 1.0 - np.tril(np.ones((S, S), dtype=np.float32))
    if mask.shape != (1, 1, S, S) or not np.array_equal(mask[0, 0], causal):
        return _numpy_fallback(x, mask, W_attn, b_attn, W_proj, b_proj)

    from concourse.bass_utils import run_bass_kernel_spmd

    if _PROGRAM is None:
        _PROGRAM = _build_program()

    in_maps = make_in_maps(x, W_attn, b_attn, W_proj)

    trace = bool(int(os.environ.get("ATTN_KERNEL_TRACE", "0")))
    res = run_bass_kernel_spmd(_PROGRAM, in_maps, list(range(N_CORES)), trace=trace)
    LAST_RESULTS = res

    y = np.zeros((B, S, N_EMBD), dtype=np.float32)
    for c in range(N_CORES):
        y[c // 2] += res.results[c]["y"]
    y += b_proj
    return y
